# revision 32
# baseline (speedup 1.0000x reference)
"""Trainium2 Bass kernel for CosyVoice3 DiT attention (B=2, S=2048, H=16, hd=64, D=1024).

Sharding: tensor parallelism over heads - 2 heads per core on 8 cores.
Each core computes QKV projections for its head slice, RoPE, full attention
for its 2 heads, then its heads' contribution to the output projection
(row-parallel). The host gather sums the 8 partial outputs and adds biases.

v3 schedule (from trace analysis of v2 @ 237us):
  - The exp stream on ACT (128 x ~1.11us) and the PE matmul stream (~140us)
    are co-critical; everything else must stay off their queues.
  - Group-boundary stalls fixed: the new group's AVs are deferred to slot ~8
    (retire policy: drain prev-group AVs 2/slot at group start, retire the
    current group's AVs only when >=9 are pending).  This gives the norm
    chain (den copy -> recip -> gpsimd broadcast -> muls) ~5 slots to clear
    the PSUM-bank WAR before the new group's first AV needs the ot bank, so
    the in-order PE queue never head-blocks -> no idle gap -> no HAM
    re-throttle to 1.2GHz.
  - Norm: reciprocal on the [1,QW] denominator BEFORE the broadcast, and no
    filler pumps in slots 0-3 so the norm chain lands in an empty DVE queue.
  - ACT queue carries ZERO DMA descriptors mid-stream (each DMA_DIRECT2D is
    ~700ns of exp-stream stall): cos/sin is one persistent tile loaded up
    front; late x chunks ride sync + gpsimd SWDGE.
  - Head: chunk-0 x split into 4x256KB DMAs across both HWDGE queues with
    per-dc-pair tiles; warmup memset reordered ahead of the vth memsets.
  - Tail: smooth AV drain (no LAG bunching), oproj copies for the last
    groups on the then-idle ACT, final output DMA split across sync+scalar.
  - q-projections of chunks 1-7 deferred into the slot stream with force
    guards; chunks 4-7 K/V spread through phases B2/C.
"""
import sys
sys.path.insert(0, "/opt/trn_rl_repo")
from collections import deque
import numpy as np

# NTFF profile hook shim: this image's antenv lacks axon_hooks, which
# bass_utils imports unconditionally when trace=True (and the boot-time
# installer degrades silently without it). Recreate the module and install
# the ctypes-based hook so neuron-profile traces work.
import types as _types
try:
    import antenv as _antenv
    if "antenv.axon_hooks" not in sys.modules:
        _hooks = _types.ModuleType("antenv.axon_hooks")
        _hook_box = [None]
        _hooks.set_axon_ntff_profile_hook = lambda h: _hook_box.__setitem__(0, h)
        _hooks.get_axon_ntff_profile_hook = lambda: _hook_box[0]
        sys.modules["antenv.axon_hooks"] = _hooks
        _antenv.axon_hooks = _hooks
        try:
            from trn_agent_boot.trn_boot import _ntff_profile_via_ctypes
            _hooks.set_axon_ntff_profile_hook(
                _ntff_profile_via_ctypes("/opt/axon/libaxon_pjrt.so"))
        except Exception:
            pass
except Exception:
    pass

import concourse.bass as bass
import concourse.mybir as mybir
from concourse import bacc
from concourse.tile import TileContext
from concourse.bass_interp import get_hw_module
from concourse import bass_utils
from concourse.masks import make_identity
bass_utils.upload_artifacts = lambda tmpdir: str(tmpdir)  # no S3 in container

# constants (hardcoded per problem spec)
B, S, D, H, HD = 2, 2048, 1024, 16, 64
T = B * S                 # 4096 tokens
NC = 8                    # cores
HPC = H // NC             # 2 heads per core
CW = HPC * HD             # 128 rows/cols per core
SCALE = 1.0 / np.sqrt(HD)
F32 = mybir.dt.float32
BF16 = mybir.dt.bfloat16
AF = mybir.ActivationFunctionType

NCHUNK = 8                # token chunks of 512
CH = T // NCHUNK          # 512
QW = 512                  # q chunk width
QC = S // QW              # 4 q chunks per batch
KT = S // 128             # 16 k-tiles per batch
DC = D // 128             # 8 contraction tiles

_CACHE = {}


def _build(use_mask: bool):
    nc = bacc.Bacc("TRN2", target_bir_lowering=False, debug=False, num_devices=NC)

    # x and the QKV weights stream as bf16: halves the dominant DMA traffic
    # and bf16 LDWEIGHTS get the FWL fast path (fp32 does not)
    xT_d = nc.dram_tensor("xt", [D, T], BF16, kind="ExternalInput")
    # host pre-arranges projection weights to [128, DC*CW] so the load is a
    # plain contiguous-row DMA
    wq_d = nc.dram_tensor("wq", [128, DC * CW], BF16, kind="ExternalInput")
    wk_d = nc.dram_tensor("wk", [128, DC * CW], BF16, kind="ExternalInput")
    wv_d = nc.dram_tensor("wv", [128, DC * CW], BF16, kind="ExternalInput")
    wo_d = nc.dram_tensor("wo", [CW, D], BF16, kind="ExternalInput")
    bq_d = nc.dram_tensor("bq", [CW, 1], F32, kind="ExternalInput")
    bk_d = nc.dram_tensor("bk", [CW, 1], F32, kind="ExternalInput")
    # cos and sign-folded sin packed side by side
    cs_d = nc.dram_tensor("cst", [CW, 2 * T], BF16, kind="ExternalInput")
    if use_mask:
        mt_d = nc.dram_tensor("maskt", [S, S], F32, kind="ExternalInput")

    ypT_d = nc.dram_tensor("ypT", [D, T], BF16, kind="ExternalOutput")

    SWAP_MASK = [i ^ 1 for i in range(32)]

    with TileContext(nc) as tc:
        with tc.tile_pool(name="persist", bufs=1) as persist, \
             tc.tile_pool(name="wpool", bufs=1) as wpool, \
             tc.tile_pool(name="xtp", bufs=16) as xtp, \
             tc.tile_pool(name="qkp", bufs=8) as qkp, \
             tc.tile_pool(name="expp", bufs=14) as expp, \
             tc.tile_pool(name="outp", bufs=4) as outp, \
             tc.tile_pool(name="yop", bufs=4) as yop, \
             tc.tile_pool(name="ps_sc", bufs=2, space="PSUM") as ps_sc, \
             tc.tile_pool(name="ps_ot", bufs=2, space="PSUM") as ps_ot, \
             tc.tile_pool(name="ps_mm", bufs=2, space="PSUM") as ps_mm:

            ident = persist.tile([128, 128], F32, name="ident")
            make_identity(nc, ident)

            # warm tile memset FIRST on the DVE queue so the HAM-warmup
            # matmuls can start ~7us in (v2 had it behind 4 vth memsets)
            wtile = persist.tile([128, CH], BF16, name="warm")
            nc.vector.memset(wtile[:, :], 0.5)

            wq = wpool.tile([128, DC, CW], BF16, name="wq_sb")
            wk = wpool.tile([128, DC, CW], BF16, name="wk_sb")
            wv = wpool.tile([128, DC, CW], BF16, name="wv_sb")
            wo = wpool.tile([CW, DC, 128], BF16, name="wo_sb")
            bq = wpool.tile([CW, 1], F32, name="bq_sb")
            bk = wpool.tile([CW, 1], F32, name="bk_sb")
            # cos/sin persistent: [128, 2, T]; one up-front load on scalar,
            # zero mid-stream DMA descriptors on the ACT queue
            csb = persist.tile([128, 2, T], BF16, name="csb")

            def emit_weight_loads():
                csview = cs_d.ap().rearrange("p (two t) -> p two t", two=2)
                # k-weights first (k-proj of chunk 0 is the first real MM);
                # cos/sin for chunks 0-1 early on scalar, the bulky rest on
                # sync behind the x prefetches (needed only from slot ~9)
                nc.scalar.dma_start(
                    out=wk, in_=wk_d.ap().rearrange("p (kc m) -> p kc m", m=CW))
                nc.scalar.dma_start(out=bk, in_=bk_d[:, :])
                nc.scalar.dma_start(
                    out=wq, in_=wq_d.ap().rearrange("p (kc m) -> p kc m", m=CW))
                nc.scalar.dma_start(out=bq, in_=bq_d[:, :])
                nc.scalar.dma_start(out=csb[:, :, 0:2 * CH], in_=csview[:, :, 0:2 * CH])
                nc.scalar.dma_start(
                    out=wv, in_=wv_d.ap().rearrange("p (kc m) -> p kc m", m=CW))
                nc.scalar.dma_start(
                    out=wo, in_=wo_d.ap().rearrange("p (mc m) -> p mc m", m=128))

            def emit_cs_rest():
                csview = cs_d.ap().rearrange("p (two t) -> p two t", two=2)
                nc.sync.dma_start(out=csb[:, :, 2 * CH:T], in_=csview[:, :, 2 * CH:T])

            qtr = persist.tile([128, T], BF16, name="qtr")    # rope'd Q^T
            ktr = persist.tile([128, T], BF16, name="ktr")    # rope'd K^T
            aoT = persist.tile([128, T], BF16, name="aoT")    # normalized attn out^T
            # V natural per chunk: [128 tok, head, ktile-in-chunk, 64+1]
            vnat = [persist.tile([128, HPC, CH // 128, HD + 1], BF16, name=f"vnat{i}")
                    for i in range(NCHUNK)]
            # V staging (transpose input): per head, double-buffered by chunk
            # parity; row 64 is the ones row for the softmax denominator.
            vth = [[persist.tile([HD + 1, CH], F32, name=f"vth{h}{p}")
                    for p in range(2)] for h in range(HPC)]
            # ones-row memsets are emitted lazily on first use (keeps the
            # DVE queue clear for chunk 0's rope chain at the head)
            vth_init = set()

            # ---- chunk x loads: 4 tiles of [128, 2(dc-pair), CH] each ----
            loaded = {}
            xview = xT_d.ap().rearrange("(dc p) t -> p dc t", p=128)

            def load_engine(n, j):
                # Sync ONLY.  A DMA_DIRECT2D that waits for a semaphore-lane
                # recycle head-blocks the engine queue that issued it: on
                # scalar that stalls the exp stream (and the scalar ring
                # measured ~2x slower anyway), on gpsimd it stalls the norm
                # broadcasts.  Sync carries nothing else, so waiting there
                # is harmless.  (SWDGE/gpsimd and scalar variants were
                # all measured slower or stall-prone - keep sync only.)
                return nc.sync

            def emit_chunk_load(n):
                if n in loaded or n >= NCHUNK:
                    return
                tcol = n * CH
                tiles = []
                for j in range(4):
                    xt = xtp.tile([128, 2, CH], BF16, name=f"x{n}j{j}", tag="xt")
                    load_engine(n, j).dma_start(
                        out=xt, in_=xview[:, 2 * j:2 * j + 2, tcol:tcol + CH])
                    tiles.append(xt)
                loaded[n] = tiles

            def xs(n, dc):
                return loaded[n][dc // 2][:, dc % 2, :]

            # ---- projection sub-generators with completion flags ----
            kdone = [False] * NCHUNK
            vdone = [False] * NCHUNK
            qdone = [False] * NCHUNK

            def qk_part(name, n, wt, bias, dst, done, on_act=False):
                emit_chunk_load(n)   # no-op if already loaded
                tcol = n * CH
                cos_c, sin_c = csb[:, 0, tcol:tcol + CH], csb[:, 1, tcol:tcol + CH]
                pp = ps_mm.tile([128, CH], F32, name=f"{name}pp{n}", tag="pp")
                for dc in range(DC):
                    nc.tensor.matmul(pp[:, :], wt[:, dc, :], xs(n, dc),
                                     start=(dc == 0), stop=(dc == DC - 1))
                    if dc % 2 == 1:
                        yield
                # bias + rope: dst = (pp+b)*cos + shuf(pp+b)*sin'
                qs = qkp.tile([128, CH], BF16, name=f"{name}s{n}", tag="qs")
                if on_act:
                    nc.scalar.activation(qs[:, :], pp[:, :], AF.Identity, bias=bias)
                else:
                    nc.vector.tensor_scalar_add(qs[:, :], pp[:, :], bias[:, :])
                qsw = qkp.tile([128, CH], BF16, name=f"{name}w{n}", tag="qs")
                nc.vector.stream_shuffle(qsw[:, :], qs[:, :], SWAP_MASK)
                yield
                t1 = qkp.tile([128, CH], BF16, name=f"{name}t1{n}", tag="qs")
                t2 = qkp.tile([128, CH], BF16, name=f"{name}t2{n}", tag="qs")
                nc.vector.tensor_mul(t1[:, :], qs[:, :], cos_c)
                nc.vector.tensor_mul(t2[:, :], qsw[:, :], sin_c)
                nc.vector.tensor_add(dst[:, tcol:tcol + CH], t1[:, :], t2[:, :])
                done[n] = True
                yield

            def v_part(n, on_act=False):
                pp = ps_mm.tile([128, CH], F32, name=f"vpp{n}", tag="pp")
                for dc in range(DC):
                    nc.tensor.matmul(pp[:, :], wv[:, dc, :], xs(n, dc),
                                     start=(dc == 0), stop=(dc == DC - 1))
                    if dc % 2 == 1 and dc < DC - 1:
                        yield
                for h in range(HPC):
                    if (h, n % 2) not in vth_init:
                        vth_init.add((h, n % 2))
                        nc.vector.memset(vth[h][n % 2][HD:HD + 1, :], 1.0)
                    if on_act:
                        nc.scalar.copy(vth[h][n % 2][0:HD, :], pp[HD * h:HD * (h + 1), :])
                    else:
                        nc.vector.tensor_copy(vth[h][n % 2][0:HD, :], pp[HD * h:HD * (h + 1), :])
                yield
                for h in range(HPC):
                    vp = ps_mm.tile([128, CH // 128, HD + 1], F32,
                                    name=f"vp{n}{h}", tag="pp")
                    for j in range(CH // 128):
                        nc.tensor.transpose(vp[:, j, :],
                                            vth[h][n % 2][:, 128 * j:128 * (j + 1)],
                                            ident[0:HD + 1, 0:HD + 1])
                    if on_act:
                        nc.scalar.copy(vnat[n][:, h, :, :], vp[:, :, :])
                    else:
                        nc.vector.tensor_copy(vnat[n][:, h, :, :], vp[:, :, :])
                    yield
                vdone[n] = True

            def kv_body(n):
                # K first (scores of this chunk's k-tiles unblock ASAP),
                # then V. Q is a separate deferred generator.
                yield from qk_part("k", n, wk, bk, ktr, kdone)
                yield from v_part(n)

            # ---- filler queues + force guards ----
            cfill = deque()    # (n, gen) chunk k+v projection generators
            qfill = deque()    # (n, gen) deferred q-projection generators
            ofill = deque()    # o-proj generators

            def pump1(q):
                while q:
                    g = q[0][1] if isinstance(q[0], tuple) else q[0]
                    try:
                        next(g)
                        return True
                    except StopIteration:
                        # trailing code after the last yield ran (flags,
                        # final DMAs) - this counts as progress
                        q.popleft()
                        return True
                return False

            def force_k(c):
                while not kdone[c]:
                    if not pump1(cfill):
                        raise RuntimeError(f"k{c} unreachable")

            def force_v(c):
                while not vdone[c]:
                    if not pump1(cfill):
                        raise RuntimeError(f"v{c} unreachable")

            def force_q(c):
                while not qdone[c]:
                    if not pump1(qfill):
                        raise RuntimeError(f"q{c} unreachable")

            # ---- attention slot pieces ----
            def emit_scores(b, qc, kt):
                toff = b * S
                qcols = slice(toff + QW * qc, toff + QW * (qc + 1))
                krows = slice(toff + 128 * kt, toff + 128 * (kt + 1))
                sc = ps_sc.tile([128, 2 * QW], F32, name=f"sc{b}{qc}{kt}", tag="sc")
                for h in range(HPC):
                    po = HD * h
                    nc.tensor.matmul(sc[:, QW * h:QW * (h + 1)],
                                     ktr[po:po + HD, krows],
                                     qtr[po:po + HD, qcols], start=True, stop=True,
                                     tile_position=(po, 0))
                if use_mask:
                    mtile = expp.tile([128, QW], F32, name=f"mt{b}{qc}{kt}", tag="mt")
                    nc.sync.dma_start(
                        out=mtile,
                        in_=mt_d[128 * kt:128 * (kt + 1), QW * qc:QW * (qc + 1)])
                    for h in range(HPC):
                        nc.vector.tensor_scalar_mul(
                            sc[:, QW * h:QW * (h + 1)], sc[:, QW * h:QW * (h + 1)], SCALE)
                        nc.vector.tensor_add(
                            sc[:, QW * h:QW * (h + 1)], sc[:, QW * h:QW * (h + 1)],
                            mtile[:, :])
                ex = expp.tile([128, 2 * QW], BF16, name=f"ex{b}{qc}{kt}", tag="ex")
                nc.scalar.activation(ex[:, :], sc[:, :], AF.Exp,
                                     scale=(1.0 if use_mask else SCALE))
                return ex

            def emit_av(b, qc, kt, ex, ots):
                toff = b * S
                cn = (toff + 128 * kt) // CH
                force_v(cn)
                j = (128 * kt % CH) // 128
                for h in range(HPC):
                    nc.tensor.matmul(ots[h][:, :], vnat[cn][:, h, j, :],
                                     ex[:, QW * h:QW * (h + 1)],
                                     start=(kt == 0), stop=(kt == KT - 1))

            def emit_norm(b, qc, ots):
                toff = b * S
                qcols = slice(toff + QW * qc, toff + QW * (qc + 1))
                # den copy + reciprocal back-to-back on DVE (no cross-engine
                # hop between them), THEN the gpsimd broadcast of the [1,QW]
                # reciprocal, then the muls.  Emitted into an empty DVE queue
                # (no fillers in slots 0-3), so the chain completes ~4 slots
                # before the new group's first AV needs the ot PSUM bank.
                dens, rc1s, rcbs = [], [], []
                for h in range(HPC):
                    den = outp.tile([1, QW], F32, name=f"den{b}{qc}{h}", tag="den")
                    nc.vector.tensor_copy(den[:, :], ots[h][HD:HD + 1, :])
                    dens.append(den)
                for h in range(HPC):
                    rc1 = outp.tile([1, QW], F32, name=f"rc1{b}{qc}{h}", tag="rc1")
                    nc.vector.reciprocal_approx_fast(rc1[:, :], dens[h][:, :])
                    rc1s.append(rc1)
                for h in range(HPC):
                    rcb = outp.tile([HD, QW], F32, name=f"rcb{b}{qc}{h}", tag="rcb")
                    nc.gpsimd.partition_broadcast(rcb[:, :], rc1s[h][:, :])
                    rcbs.append(rcb)
                for h in range(HPC):
                    po = HD * h
                    nc.vector.tensor_mul(aoT[po:po + HD, qcols],
                                         ots[h][0:HD, :], rcbs[h][:, :])

            def oproj_gen(b, qc, act_half=False, split4=False):
                toff = b * S
                qcols = slice(toff + QW * qc, toff + QW * (qc + 1))
                yog = yop.tile([128, DC, QW], BF16, name=f"yog{b}{qc}", tag="yog")
                yview = ypT_d.ap().rearrange("(mc p) t -> p mc t", p=128)
                for mo in range(DC):
                    yp = ps_mm.tile([128, QW], F32, name=f"yp{b}{qc}{mo}", tag="pp")
                    nc.tensor.matmul(yp[:, :], wo[:, mo, :], aoT[:, qcols],
                                     start=True, stop=True)
                    if act_half and mo % 2 == 1:
                        nc.scalar.copy(yog[:, mo, :], yp[:, :])
                    else:
                        nc.vector.tensor_copy(yog[:, mo, :], yp[:, :])
                    if split4:
                        # tail: 4 finer DMAs alternating sync/scalar so the
                        # last transfer starts as early as possible
                        if mo % 2 == 1:
                            eng = nc.sync if (mo // 2) % 2 == 0 else nc.scalar
                            eng.dma_start(out=yview[:, mo - 1:mo + 1, qcols],
                                          in_=yog[:, mo - 1:mo + 1, :])
                    elif mo == DC // 2 - 1:
                        nc.sync.dma_start(out=yview[:, 0:DC // 2, qcols],
                                          in_=yog[:, 0:DC // 2, :])
                    yield
                if not split4:
                    nc.sync.dma_start(out=yview[:, DC // 2:DC, qcols],
                                      in_=yog[:, DC // 2:DC, :])

            def drain(g):
                for _ in g:
                    pass

            # ---- emission schedule ----
            # group sequence and pend-based AV retirement
            groups = [(b, qc) for b in range(2) for qc in range(QC)]
            pend = deque()   # (seq, b, qc, kt, ex, ots)

            def retire_one():
                if not pend:
                    return False
                seq, pb, pqc, pkt, pex, pots = pend.popleft()
                emit_av(pb, pqc, pkt, pex, pots)
                if pkt == KT - 1:
                    emit_norm(pb, pqc, pots)
                    last = (pb == 1 and pqc == QC - 1)
                    ofill.append(oproj_gen(pb, pqc, act_half=last, split4=last))
                return True

            def new_ots(b, qc):
                return [ps_ot.tile([HD + 1, QW], F32, name=f"ot{b}{qc}{h}", tag="ot")
                        for h in range(HPC)]

            # prolog: chunk 0-3 x loads (FIFO behind each other on the two
            # HWDGE rings; chunk 3 lands ~17us, well before its slot-12
            # deadline), weights, warmup, chunk0 projections
            emit_chunk_load(0)
            emit_weight_loads()
            emit_chunk_load(1)
            wps = ps_mm.tile([128, CH], F32, name="warmps", tag="pp")
            for _ in range(10):
                nc.tensor.matmul(wps[:, :], wtile[:, 0:128], wtile[:, :],
                                 start=True, stop=True)
            emit_chunk_load(2)
            emit_cs_rest()
            drain(qk_part("k", 0, wk, bk, ktr, kdone, on_act=True))
            emit_chunk_load(3)
            drain(qk_part("q", 0, wq, bq, qtr, qdone, on_act=True))
            # v0 is only needed by the first AV at slot ~10: pump it in the
            # slot stream instead of serializing it before the first scores
            cfill.append((0, v_part(0)))
            for n in range(1, NCHUNK):
                cfill.append((n, kv_body(n)))
                qfill.append((n, qk_part("q", n, wq, bq, qtr, qdone)))
            qdone[0] = True

            # ---- main slot loop ----
            for seq, (b, qc) in enumerate(groups):
                force_q(4 * b + qc)
                ots = new_ots(b, qc)
                for i in range(KT):
                    # late x prefetch: chunks 4-7 all feed group (1,0), so
                    # their loads go out during (0,1)/(0,2)
                    if seq == 1 and i == 4:
                        emit_chunk_load(4)
                    elif seq == 1 and i == 12:
                        emit_chunk_load(5)
                    elif seq == 2 and i == 4:
                        emit_chunk_load(6)
                    elif seq == 2 and i == 12:
                        emit_chunk_load(7)
                    kchunk = (b * S + 128 * i) // CH
                    force_k(kchunk)
                    ex = emit_scores(b, qc, i)
                    pend.append((seq, b, qc, i, ex, ots))
                    # retire policy: drain prev-group AVs 2/slot; defer the
                    # current group's AVs until >=11 pending (slot ~10, so
                    # the prev norm chain has cleared the ot-bank WAR with
                    # margin).  Last group retires 2/slot to shorten the
                    # serial drain tail.
                    drained = 0
                    while pend and pend[0][0] < seq and drained < 2:
                        retire_one()
                        drained += 1
                    if drained == 0 and len(pend) >= 11:
                        retire_one()
                        if seq == len(groups) - 1 and len(pend) >= 11:
                            retire_one()
                    # fillers: none in slots 0-3 of seq>0 (keep DVE clear
                    # for the prev group's norm chain), and oproj only in
                    # slots 7-15 (its CASTs otherwise queue ahead of the
                    # norm ops on DVE).  Chunks 4-7 k+v (~17us) must all
                    # land by slot ~64 (group (1,0) spans kt of chunks
                    # 4-7), so phases A/B pump cfill at 2-3 yields/slot.
                    # seq==7 reserves its ofill backlog to feed the PE
                    # through the serial drain tail.
                    if seq == 0:
                        pump1(cfill)
                        pump1(cfill)
                        if i % 2 == 1:
                            pump1(qfill)
                        else:
                            pump1(cfill)
                    elif i >= 4:
                        if i >= 7 and seq < 7:
                            pump1(ofill)
                        if 4 <= i <= 6:
                            # ofill-free window: finish the next group's
                            # q-part while the DVE queue is short
                            pump1(qfill)
                        if i % 2 == 0:
                            if not pump1(cfill):
                                pump1(qfill)
                        else:
                            if not pump1(qfill):
                                pump1(cfill)
                        if seq <= 3:
                            pump1(cfill)

            # ---- drain ----
            while pend:
                retire_one()
                pump1(ofill)
            while ofill:
                pump1(ofill)
            while cfill:
                pump1(cfill)
            while qfill:
                pump1(qfill)

    nc.compile()
    nc.m = get_hw_module(nc.m)
    return nc


def _get_nc(use_mask: bool):
    key = ("nc", use_mask)
    if key not in _CACHE:
        _CACHE[key] = _build(use_mask)
    return _CACHE[key]


def kernel(x, rope, mask, Wq, bq, Wk, bk, Wv, bv, Wo, bo, _trace=False):
    import ml_dtypes
    x = np.asarray(x, dtype=np.float32)
    rope = np.asarray(rope, dtype=np.float32)
    mask = np.asarray(mask, dtype=np.float32)
    Wq = np.asarray(Wq, dtype=np.float32)
    Wk = np.asarray(Wk, dtype=np.float32)
    Wv = np.asarray(Wv, dtype=np.float32)
    Wo = np.asarray(Wo, dtype=np.float32)
    use_mask = bool(np.any(mask))

    xT = np.ascontiguousarray(x.reshape(T, D).T).astype(ml_dtypes.bfloat16)
    cos = rope[0, 0, :, 0, :]                                     # [S, 64]
    sin = rope[1, 0, :, 0, :]
    sgn = np.where(np.arange(HD) % 2 == 0, -1.0, 1.0).astype(np.float32)[:, None]
    cosT = np.tile(cos.T, (HPC, B))
    sinT = np.tile(sin.T * sgn, (HPC, B))
    csT = np.ascontiguousarray(
        np.concatenate([cosT, sinT], axis=1)).astype(ml_dtypes.bfloat16)

    nc = _get_nc(use_mask)

    def warr(W, cs):
        # [D, CW] -> [128, DC*CW]: partition p holds rows p, 128+p, ... so
        # the device DMA is a contiguous-row load
        return np.ascontiguousarray(
            W[:, cs].reshape(DC, 128, CW).transpose(1, 0, 2)
            .reshape(128, DC * CW)).astype(ml_dtypes.bfloat16)

    in_maps = []
    for c in range(NC):
        cs = slice(CW * c, CW * (c + 1))
        m = dict(
            xt=xT,
            wq=warr(Wq, cs),
            bq=np.ascontiguousarray(bq[cs]).reshape(CW, 1).astype(np.float32),
            wk=warr(Wk, cs),
            bk=np.ascontiguousarray(bk[cs]).reshape(CW, 1).astype(np.float32),
            wv=warr(Wv, cs),
            wo=np.ascontiguousarray(Wo[cs, :]).astype(ml_dtypes.bfloat16),
            cst=csT,
        )
        if use_mask:
            m["maskt"] = np.ascontiguousarray(mask[0, 0].T).astype(np.float32)
        in_maps.append(m)

    # transient device wedges (NRT_EXEC_UNIT_UNRECOVERABLE) clear on retry
    last_err = None
    for _attempt in range(3):
        try:
            res = bass_utils.run_bass_kernel_spmd(
                nc, in_maps, core_ids=list(range(NC)), trace=_trace)
            break
        except Exception as e:  # noqa: BLE001
            last_err = e
            import time as _time
            _time.sleep(2.0)
    else:
        raise last_err
    # row-parallel unshard: sum the per-core bf16 partials in fp32, add the
    # output bias and the folded V bias (bv commutes through attention).
    ypT = res.results[0]["ypT"].astype(np.float32)
    for c in range(1, NC):
        ypT = ypT + res.results[c]["ypT"].astype(np.float32)
    bo_eff = np.asarray(bo, dtype=np.float32) + \
        np.asarray(bv, dtype=np.float32) @ Wo
    out = (ypT.T + bo_eff).reshape(B, S, D).astype(np.float32)
    out = np.ascontiguousarray(out)
    if _trace:
        return out, res
    return out


# revision 33
# speedup vs baseline: 1.2104x; 1.2104x over previous
"""Trainium2 Bass kernel for CosyVoice3 DiT attention (B=2, S=2048, H=16, hd=64, D=1024).

Sharding: tensor parallelism over heads - 2 heads per core on 8 cores.
Each core computes QKV projections for its head slice, RoPE, full attention
for its 2 heads, then its heads' contribution to the output projection
(row-parallel). The host gather sums the 8 partial outputs and adds biases.

v3 schedule (from trace analysis of v2 @ 237us):
  - The exp stream on ACT (128 x ~1.11us) and the PE matmul stream (~140us)
    are co-critical; everything else must stay off their queues.
  - Group-boundary stalls fixed: the new group's AVs are deferred to slot ~8
    (retire policy: drain prev-group AVs 2/slot at group start, retire the
    current group's AVs only when >=9 are pending).  This gives the norm
    chain (den copy -> recip -> gpsimd broadcast -> muls) ~5 slots to clear
    the PSUM-bank WAR before the new group's first AV needs the ot bank, so
    the in-order PE queue never head-blocks -> no idle gap -> no HAM
    re-throttle to 1.2GHz.
  - Norm: reciprocal on the [1,QW] denominator BEFORE the broadcast, and no
    filler pumps in slots 0-3 so the norm chain lands in an empty DVE queue.
  - ACT queue carries ZERO DMA descriptors mid-stream (each DMA_DIRECT2D is
    ~700ns of exp-stream stall): cos/sin is one persistent tile loaded up
    front; late x chunks ride sync + gpsimd SWDGE.
  - Head: chunk-0 x split into 4x256KB DMAs across both HWDGE queues with
    per-dc-pair tiles; warmup memset reordered ahead of the vth memsets.
  - Tail: smooth AV drain (no LAG bunching), oproj copies for the last
    groups on the then-idle ACT, final output DMA split across sync+scalar.
  - q-projections of chunks 1-7 deferred into the slot stream with force
    guards; chunks 4-7 K/V spread through phases B2/C.
"""
import sys
sys.path.insert(0, "/opt/trn_rl_repo")
from collections import deque
import numpy as np

# NTFF profile hook shim: this image's antenv lacks axon_hooks, which
# bass_utils imports unconditionally when trace=True (and the boot-time
# installer degrades silently without it). Recreate the module and install
# the ctypes-based hook so neuron-profile traces work.
import types as _types
try:
    import antenv as _antenv
    if "antenv.axon_hooks" not in sys.modules:
        _hooks = _types.ModuleType("antenv.axon_hooks")
        _hook_box = [None]
        _hooks.set_axon_ntff_profile_hook = lambda h: _hook_box.__setitem__(0, h)
        _hooks.get_axon_ntff_profile_hook = lambda: _hook_box[0]
        sys.modules["antenv.axon_hooks"] = _hooks
        _antenv.axon_hooks = _hooks
        try:
            from trn_agent_boot.trn_boot import _ntff_profile_via_ctypes
            _hooks.set_axon_ntff_profile_hook(
                _ntff_profile_via_ctypes("/opt/axon/libaxon_pjrt.so"))
        except Exception:
            pass
except Exception:
    pass

import concourse.bass as bass
import concourse.mybir as mybir
from concourse import bacc
from concourse.tile import TileContext
from concourse.bass_interp import get_hw_module
from concourse import bass_utils
from concourse.masks import make_identity
bass_utils.upload_artifacts = lambda tmpdir: str(tmpdir)  # no S3 in container

# constants (hardcoded per problem spec)
B, S, D, H, HD = 2, 2048, 1024, 16, 64
T = B * S                 # 4096 tokens
NC = 8                    # cores
HPC = H // NC             # 2 heads per core
CW = HPC * HD             # 128 rows/cols per core
SCALE = 1.0 / np.sqrt(HD)
F32 = mybir.dt.float32
BF16 = mybir.dt.bfloat16
AF = mybir.ActivationFunctionType

NCHUNK = 8                # token chunks of 512
CH = T // NCHUNK          # 512
QW = 512                  # q chunk width
QC = S // QW              # 4 q chunks per batch
KT = S // 128             # 16 k-tiles per batch
DC = D // 128             # 8 contraction tiles

_CACHE = {}


def _build(use_mask: bool):
    nc = bacc.Bacc("TRN2", target_bir_lowering=False, debug=False, num_devices=NC)

    # x and the QKV weights stream as bf16: halves the dominant DMA traffic
    # and bf16 LDWEIGHTS get the FWL fast path (fp32 does not)
    xT_d = nc.dram_tensor("xt", [D, T], BF16, kind="ExternalInput")
    # host pre-arranges projection weights to [128, DC*CW] so the load is a
    # plain contiguous-row DMA
    wq_d = nc.dram_tensor("wq", [128, DC * CW], BF16, kind="ExternalInput")
    wk_d = nc.dram_tensor("wk", [128, DC * CW], BF16, kind="ExternalInput")
    wv_d = nc.dram_tensor("wv", [128, DC * CW], BF16, kind="ExternalInput")
    wo_d = nc.dram_tensor("wo", [CW, D], BF16, kind="ExternalInput")
    bq_d = nc.dram_tensor("bq", [CW, 1], F32, kind="ExternalInput")
    bk_d = nc.dram_tensor("bk", [CW, 1], F32, kind="ExternalInput")
    # cos and sign-folded sin packed side by side
    cs_d = nc.dram_tensor("cst", [CW, 2 * T], BF16, kind="ExternalInput")
    if use_mask:
        mt_d = nc.dram_tensor("maskt", [S, S], F32, kind="ExternalInput")

    ypT_d = nc.dram_tensor("ypT", [D, T], BF16, kind="ExternalOutput")

    SWAP_MASK = [i ^ 1 for i in range(32)]

    with TileContext(nc) as tc:
        with tc.tile_pool(name="persist", bufs=1) as persist, \
             tc.tile_pool(name="wpool", bufs=1) as wpool, \
             tc.tile_pool(name="xtp", bufs=16) as xtp, \
             tc.tile_pool(name="qkp", bufs=8) as qkp, \
             tc.tile_pool(name="expp", bufs=14) as expp, \
             tc.tile_pool(name="outp", bufs=4) as outp, \
             tc.tile_pool(name="yop", bufs=4) as yop, \
             tc.tile_pool(name="ps_sc", bufs=2, space="PSUM") as ps_sc, \
             tc.tile_pool(name="ps_ot", bufs=2, space="PSUM") as ps_ot, \
             tc.tile_pool(name="ps_mm", bufs=2, space="PSUM") as ps_mm:

            ident = persist.tile([128, 128], F32, name="ident")
            make_identity(nc, ident)

            # warm tile memset FIRST on the DVE queue so the HAM-warmup
            # matmuls can start ~7us in (v2 had it behind 4 vth memsets)
            wtile = persist.tile([128, CH], BF16, name="warm")
            nc.vector.memset(wtile[:, :], 0.5)

            wq = wpool.tile([128, DC, CW], BF16, name="wq_sb")
            wk = wpool.tile([128, DC, CW], BF16, name="wk_sb")
            wv = wpool.tile([128, DC, CW], BF16, name="wv_sb")
            wo = wpool.tile([CW, DC, 128], BF16, name="wo_sb")
            bq = wpool.tile([CW, 1], F32, name="bq_sb")
            bk = wpool.tile([CW, 1], F32, name="bk_sb")
            # cos/sin persistent: [128, 2, T]; one up-front load on scalar,
            # zero mid-stream DMA descriptors on the ACT queue
            csb = persist.tile([128, 2, T], BF16, name="csb")

            def emit_weight_loads():
                csview = cs_d.ap().rearrange("p (two t) -> p two t", two=2)
                # k-weights first (k-proj of chunk 0 is the first real MM);
                # cos/sin for chunks 0-1 early on scalar, the bulky rest on
                # sync behind the x prefetches (needed only from slot ~9)
                nc.scalar.dma_start(
                    out=wk, in_=wk_d.ap().rearrange("p (kc m) -> p kc m", m=CW))
                nc.scalar.dma_start(out=bk, in_=bk_d[:, :])
                nc.scalar.dma_start(
                    out=wq, in_=wq_d.ap().rearrange("p (kc m) -> p kc m", m=CW))
                nc.scalar.dma_start(out=bq, in_=bq_d[:, :])
                nc.scalar.dma_start(out=csb[:, :, 0:2 * CH], in_=csview[:, :, 0:2 * CH])
                nc.scalar.dma_start(
                    out=wv, in_=wv_d.ap().rearrange("p (kc m) -> p kc m", m=CW))
                nc.scalar.dma_start(
                    out=wo, in_=wo_d.ap().rearrange("p (mc m) -> p mc m", m=128))

            def emit_cs_rest():
                csview = cs_d.ap().rearrange("p (two t) -> p two t", two=2)
                nc.sync.dma_start(out=csb[:, :, 2 * CH:T], in_=csview[:, :, 2 * CH:T])

            qtr = persist.tile([128, T], BF16, name="qtr")    # rope'd Q^T
            ktr = persist.tile([128, T], BF16, name="ktr")    # rope'd K^T
            aoT = persist.tile([128, T], BF16, name="aoT")    # normalized attn out^T
            # V natural per chunk: [128 tok, head, ktile-in-chunk, 64+1]
            vnat = [persist.tile([128, HPC, CH // 128, HD + 1], BF16, name=f"vnat{i}")
                    for i in range(NCHUNK)]
            # V staging (transpose input): per head, double-buffered by chunk
            # parity; row 64 is the ones row for the softmax denominator.
            vth = [[persist.tile([HD + 1, CH], F32, name=f"vth{h}{p}")
                    for p in range(2)] for h in range(HPC)]
            # ones-row memsets are emitted lazily on first use (keeps the
            # DVE queue clear for chunk 0's rope chain at the head)
            vth_init = set()

            # ---- chunk x loads: 4 tiles of [128, 2(dc-pair), CH] each ----
            loaded = {}
            xview = xT_d.ap().rearrange("(dc p) t -> p dc t", p=128)

            def load_engine(n, j):
                # Sync ONLY.  A DMA_DIRECT2D that waits for a semaphore-lane
                # recycle head-blocks the engine queue that issued it: on
                # scalar that stalls the exp stream (and the scalar ring
                # measured ~2x slower anyway), on gpsimd it stalls the norm
                # broadcasts.  Sync carries nothing else, so waiting there
                # is harmless.  (SWDGE/gpsimd and scalar variants were
                # all measured slower or stall-prone - keep sync only.)
                return nc.sync

            def emit_chunk_load(n):
                if n in loaded or n >= NCHUNK:
                    return
                tcol = n * CH
                tiles = []
                for j in range(4):
                    xt = xtp.tile([128, 2, CH], BF16, name=f"x{n}j{j}", tag="xt")
                    load_engine(n, j).dma_start(
                        out=xt, in_=xview[:, 2 * j:2 * j + 2, tcol:tcol + CH])
                    tiles.append(xt)
                loaded[n] = tiles

            def xs(n, dc):
                return loaded[n][dc // 2][:, dc % 2, :]

            # ---- projection sub-generators with completion flags ----
            kdone = [False] * NCHUNK
            vdone = [False] * NCHUNK
            qdone = [False] * NCHUNK

            def qk_part(name, n, wt, bias, dst, done, on_act=False):
                emit_chunk_load(n)   # no-op if already loaded
                tcol = n * CH
                cos_c, sin_c = csb[:, 0, tcol:tcol + CH], csb[:, 1, tcol:tcol + CH]
                pp = ps_mm.tile([128, CH], F32, name=f"{name}pp{n}", tag="pp")
                for dc in range(DC):
                    nc.tensor.matmul(pp[:, :], wt[:, dc, :], xs(n, dc),
                                     start=(dc == 0), stop=(dc == DC - 1))
                    if dc % 2 == 1:
                        yield
                # bias + rope: dst = (pp+b)*cos + shuf(pp+b)*sin'
                qs = qkp.tile([128, CH], BF16, name=f"{name}s{n}", tag="qs")
                if on_act:
                    nc.scalar.activation(qs[:, :], pp[:, :], AF.Identity, bias=bias)
                else:
                    nc.vector.tensor_scalar_add(qs[:, :], pp[:, :], bias[:, :])
                qsw = qkp.tile([128, CH], BF16, name=f"{name}w{n}", tag="qs")
                nc.vector.stream_shuffle(qsw[:, :], qs[:, :], SWAP_MASK)
                yield
                t1 = qkp.tile([128, CH], BF16, name=f"{name}t1{n}", tag="qs")
                t2 = qkp.tile([128, CH], BF16, name=f"{name}t2{n}", tag="qs")
                nc.vector.tensor_mul(t1[:, :], qs[:, :], cos_c)
                nc.vector.tensor_mul(t2[:, :], qsw[:, :], sin_c)
                nc.vector.tensor_add(dst[:, tcol:tcol + CH], t1[:, :], t2[:, :])
                done[n] = True
                yield

            def v_part(n, on_act=False):
                pp = ps_mm.tile([128, CH], F32, name=f"vpp{n}", tag="pp")
                for dc in range(DC):
                    nc.tensor.matmul(pp[:, :], wv[:, dc, :], xs(n, dc),
                                     start=(dc == 0), stop=(dc == DC - 1))
                    if dc % 2 == 1 and dc < DC - 1:
                        yield
                for h in range(HPC):
                    if (h, n % 2) not in vth_init:
                        vth_init.add((h, n % 2))
                        nc.vector.memset(vth[h][n % 2][HD:HD + 1, :], 1.0)
                    if on_act:
                        nc.scalar.copy(vth[h][n % 2][0:HD, :], pp[HD * h:HD * (h + 1), :])
                    else:
                        nc.vector.tensor_copy(vth[h][n % 2][0:HD, :], pp[HD * h:HD * (h + 1), :])
                yield
                for h in range(HPC):
                    vp = ps_mm.tile([128, CH // 128, HD + 1], F32,
                                    name=f"vp{n}{h}", tag="pp")
                    for j in range(CH // 128):
                        nc.tensor.transpose(vp[:, j, :],
                                            vth[h][n % 2][:, 128 * j:128 * (j + 1)],
                                            ident[0:HD + 1, 0:HD + 1])
                    if on_act:
                        nc.scalar.copy(vnat[n][:, h, :, :], vp[:, :, :])
                    else:
                        nc.vector.tensor_copy(vnat[n][:, h, :, :], vp[:, :, :])
                    yield
                vdone[n] = True

            def kv_body(n):
                # K first (scores of this chunk's k-tiles unblock ASAP),
                # then V. Q is a separate deferred generator.
                yield from qk_part("k", n, wk, bk, ktr, kdone)
                yield from v_part(n)

            # ---- filler queues + force guards ----
            cfill = deque()    # (n, gen) chunk k+v projection generators
            qfill = deque()    # (n, gen) deferred q-projection generators
            ofill = deque()    # o-proj generators

            def pump1(q):
                while q:
                    g = q[0][1] if isinstance(q[0], tuple) else q[0]
                    try:
                        next(g)
                        return True
                    except StopIteration:
                        # trailing code after the last yield ran (flags,
                        # final DMAs) - this counts as progress
                        q.popleft()
                        return True
                return False

            def force_k(c):
                while not kdone[c]:
                    if not pump1(cfill):
                        raise RuntimeError(f"k{c} unreachable")

            def force_v(c):
                while not vdone[c]:
                    if not pump1(cfill):
                        raise RuntimeError(f"v{c} unreachable")

            def force_q(c):
                while not qdone[c]:
                    if not pump1(qfill):
                        raise RuntimeError(f"q{c} unreachable")

            # ---- attention slot pieces ----
            def emit_scores(b, qc, kt):
                toff = b * S
                qcols = slice(toff + QW * qc, toff + QW * (qc + 1))
                krows = slice(toff + 128 * kt, toff + 128 * (kt + 1))
                sc = ps_sc.tile([128, 2 * QW], F32, name=f"sc{b}{qc}{kt}", tag="sc")
                for h in range(HPC):
                    po = HD * h
                    nc.tensor.matmul(sc[:, QW * h:QW * (h + 1)],
                                     ktr[po:po + HD, krows],
                                     qtr[po:po + HD, qcols], start=True, stop=True,
                                     tile_position=(po, 0))
                if use_mask:
                    mtile = expp.tile([128, QW], F32, name=f"mt{b}{qc}{kt}", tag="mt")
                    nc.sync.dma_start(
                        out=mtile,
                        in_=mt_d[128 * kt:128 * (kt + 1), QW * qc:QW * (qc + 1)])
                    for h in range(HPC):
                        nc.vector.tensor_scalar_mul(
                            sc[:, QW * h:QW * (h + 1)], sc[:, QW * h:QW * (h + 1)], SCALE)
                        nc.vector.tensor_add(
                            sc[:, QW * h:QW * (h + 1)], sc[:, QW * h:QW * (h + 1)],
                            mtile[:, :])
                ex = expp.tile([128, 2 * QW], BF16, name=f"ex{b}{qc}{kt}", tag="ex")
                nc.scalar.activation(ex[:, :], sc[:, :], AF.Exp,
                                     scale=(1.0 if use_mask else SCALE))
                return ex

            def emit_av(b, qc, kt, ex, ots):
                toff = b * S
                cn = (toff + 128 * kt) // CH
                force_v(cn)
                j = (128 * kt % CH) // 128
                for h in range(HPC):
                    nc.tensor.matmul(ots[h][:, :], vnat[cn][:, h, j, :],
                                     ex[:, QW * h:QW * (h + 1)],
                                     start=(kt == 0), stop=(kt == KT - 1))

            def emit_norm(b, qc, ots):
                toff = b * S
                qcols = slice(toff + QW * qc, toff + QW * (qc + 1))
                # den copy + reciprocal back-to-back on DVE (no cross-engine
                # hop between them), THEN the gpsimd broadcast of the [1,QW]
                # reciprocal, then the muls.  Emitted into an empty DVE queue
                # (no fillers in slots 0-3), so the chain completes ~4 slots
                # before the new group's first AV needs the ot PSUM bank.
                dens, rc1s, rcbs = [], [], []
                for h in range(HPC):
                    den = outp.tile([1, QW], F32, name=f"den{b}{qc}{h}", tag="den")
                    nc.vector.tensor_copy(den[:, :], ots[h][HD:HD + 1, :])
                    dens.append(den)
                for h in range(HPC):
                    rc1 = outp.tile([1, QW], F32, name=f"rc1{b}{qc}{h}", tag="rc1")
                    nc.vector.reciprocal_approx_fast(rc1[:, :], dens[h][:, :])
                    rc1s.append(rc1)
                for h in range(HPC):
                    rcb = outp.tile([HD, QW], F32, name=f"rcb{b}{qc}{h}", tag="rcb")
                    nc.gpsimd.partition_broadcast(rcb[:, :], rc1s[h][:, :])
                    rcbs.append(rcb)
                for h in range(HPC):
                    po = HD * h
                    nc.vector.tensor_mul(aoT[po:po + HD, qcols],
                                         ots[h][0:HD, :], rcbs[h][:, :])

            def oproj_gen(b, qc, act_half=False, split4=False):
                toff = b * S
                qcols = slice(toff + QW * qc, toff + QW * (qc + 1))
                yog = yop.tile([128, DC, QW], BF16, name=f"yog{b}{qc}", tag="yog")
                yview = ypT_d.ap().rearrange("(mc p) t -> p mc t", p=128)
                for mo in range(DC):
                    yp = ps_mm.tile([128, QW], F32, name=f"yp{b}{qc}{mo}", tag="pp")
                    nc.tensor.matmul(yp[:, :], wo[:, mo, :], aoT[:, qcols],
                                     start=True, stop=True)
                    if act_half and mo % 2 == 1:
                        nc.scalar.copy(yog[:, mo, :], yp[:, :])
                    else:
                        nc.vector.tensor_copy(yog[:, mo, :], yp[:, :])
                    if split4:
                        # tail: 4 finer DMAs alternating sync/scalar so the
                        # last transfer starts as early as possible
                        if mo % 2 == 1:
                            eng = nc.sync if (mo // 2) % 2 == 0 else nc.scalar
                            eng.dma_start(out=yview[:, mo - 1:mo + 1, qcols],
                                          in_=yog[:, mo - 1:mo + 1, :])
                    elif mo == DC // 2 - 1:
                        nc.sync.dma_start(out=yview[:, 0:DC // 2, qcols],
                                          in_=yog[:, 0:DC // 2, :])
                    yield
                if not split4:
                    nc.sync.dma_start(out=yview[:, DC // 2:DC, qcols],
                                      in_=yog[:, DC // 2:DC, :])

            def drain(g):
                for _ in g:
                    pass

            # ---- emission schedule ----
            # group sequence and pend-based AV retirement
            groups = [(b, qc) for b in range(2) for qc in range(QC)]
            pend = deque()   # (seq, b, qc, kt, ex, ots)

            def retire_one():
                if not pend:
                    return False
                seq, pb, pqc, pkt, pex, pots = pend.popleft()
                emit_av(pb, pqc, pkt, pex, pots)
                if pkt == KT - 1:
                    emit_norm(pb, pqc, pots)
                    last = (pb == 1 and pqc == QC - 1)
                    ofill.append(oproj_gen(pb, pqc, act_half=last, split4=last))
                return True

            def new_ots(b, qc):
                return [ps_ot.tile([HD + 1, QW], F32, name=f"ot{b}{qc}{h}", tag="ot")
                        for h in range(HPC)]

            # prolog: chunk 0-3 x loads (FIFO behind each other on the two
            # HWDGE rings; chunk 3 lands ~17us, well before its slot-12
            # deadline), weights, warmup, chunk0 projections
            emit_chunk_load(0)
            emit_weight_loads()
            emit_chunk_load(1)
            wps = ps_mm.tile([128, CH], F32, name="warmps", tag="pp")
            for _ in range(10):
                nc.tensor.matmul(wps[:, :], wtile[:, 0:128], wtile[:, :],
                                 start=True, stop=True)
            emit_chunk_load(2)
            emit_cs_rest()
            drain(qk_part("k", 0, wk, bk, ktr, kdone, on_act=True))
            emit_chunk_load(3)
            drain(qk_part("q", 0, wq, bq, qtr, qdone, on_act=True))
            # v0 is only needed by the first AV at slot ~10: pump it in the
            # slot stream instead of serializing it before the first scores
            cfill.append((0, v_part(0)))
            for n in range(1, NCHUNK):
                cfill.append((n, kv_body(n)))
                qfill.append((n, qk_part("q", n, wq, bq, qtr, qdone)))
            qdone[0] = True

            # ---- main slot loop ----
            for seq, (b, qc) in enumerate(groups):
                force_q(4 * b + qc)
                ots = new_ots(b, qc)
                for i in range(KT):
                    # late x prefetch: chunks 4-7 all feed group (1,0), so
                    # their loads go out during (0,1)/(0,2)
                    if seq == 1 and i == 4:
                        emit_chunk_load(4)
                    elif seq == 1 and i == 12:
                        emit_chunk_load(5)
                    elif seq == 2 and i == 4:
                        emit_chunk_load(6)
                    elif seq == 2 and i == 12:
                        emit_chunk_load(7)
                    kchunk = (b * S + 128 * i) // CH
                    force_k(kchunk)
                    ex = emit_scores(b, qc, i)
                    pend.append((seq, b, qc, i, ex, ots))
                    # retire policy: drain prev-group AVs 2/slot; defer the
                    # current group's AVs until >=11 pending (slot ~10, so
                    # the prev norm chain has cleared the ot-bank WAR with
                    # margin).  Last group retires 2/slot to shorten the
                    # serial drain tail.
                    drained = 0
                    while pend and pend[0][0] < seq and drained < 2:
                        retire_one()
                        drained += 1
                    if drained == 0 and len(pend) >= 11:
                        retire_one()
                        if seq == len(groups) - 1 and len(pend) >= 11:
                            retire_one()
                    # fillers: none in slots 0-3 of seq>0 (keep DVE clear
                    # for the prev group's norm chain), and oproj only in
                    # slots 7-15 (its CASTs otherwise queue ahead of the
                    # norm ops on DVE).  Chunks 4-7 k+v (~17us) must all
                    # land by slot ~64 (group (1,0) spans kt of chunks
                    # 4-7), so phases A/B pump cfill at 2-3 yields/slot.
                    # seq==7 reserves its ofill backlog to feed the PE
                    # through the serial drain tail.
                    if seq == 0:
                        pump1(cfill)
                        pump1(cfill)
                        if i % 2 == 1:
                            pump1(qfill)
                        else:
                            pump1(cfill)
                    elif i >= 4:
                        if i >= 7 and seq < 7:
                            pump1(ofill)
                        if i % 2 == 0:
                            if not pump1(cfill):
                                pump1(qfill)
                        else:
                            if not pump1(qfill):
                                pump1(cfill)
                        if seq <= 3:
                            pump1(cfill)

            # ---- drain ----
            while pend:
                retire_one()
                pump1(ofill)
            while ofill:
                pump1(ofill)
            while cfill:
                pump1(cfill)
            while qfill:
                pump1(qfill)

    nc.compile()
    nc.m = get_hw_module(nc.m)
    return nc


def _get_nc(use_mask: bool):
    key = ("nc", use_mask)
    if key not in _CACHE:
        _CACHE[key] = _build(use_mask)
    return _CACHE[key]


def kernel(x, rope, mask, Wq, bq, Wk, bk, Wv, bv, Wo, bo, _trace=False):
    import ml_dtypes
    x = np.asarray(x, dtype=np.float32)
    rope = np.asarray(rope, dtype=np.float32)
    mask = np.asarray(mask, dtype=np.float32)
    Wq = np.asarray(Wq, dtype=np.float32)
    Wk = np.asarray(Wk, dtype=np.float32)
    Wv = np.asarray(Wv, dtype=np.float32)
    Wo = np.asarray(Wo, dtype=np.float32)
    use_mask = bool(np.any(mask))

    xT = np.ascontiguousarray(x.reshape(T, D).T).astype(ml_dtypes.bfloat16)
    cos = rope[0, 0, :, 0, :]                                     # [S, 64]
    sin = rope[1, 0, :, 0, :]
    sgn = np.where(np.arange(HD) % 2 == 0, -1.0, 1.0).astype(np.float32)[:, None]
    cosT = np.tile(cos.T, (HPC, B))
    sinT = np.tile(sin.T * sgn, (HPC, B))
    csT = np.ascontiguousarray(
        np.concatenate([cosT, sinT], axis=1)).astype(ml_dtypes.bfloat16)

    nc = _get_nc(use_mask)

    def warr(W, cs):
        # [D, CW] -> [128, DC*CW]: partition p holds rows p, 128+p, ... so
        # the device DMA is a contiguous-row load
        return np.ascontiguousarray(
            W[:, cs].reshape(DC, 128, CW).transpose(1, 0, 2)
            .reshape(128, DC * CW)).astype(ml_dtypes.bfloat16)

    in_maps = []
    for c in range(NC):
        cs = slice(CW * c, CW * (c + 1))
        m = dict(
            xt=xT,
            wq=warr(Wq, cs),
            bq=np.ascontiguousarray(bq[cs]).reshape(CW, 1).astype(np.float32),
            wk=warr(Wk, cs),
            bk=np.ascontiguousarray(bk[cs]).reshape(CW, 1).astype(np.float32),
            wv=warr(Wv, cs),
            wo=np.ascontiguousarray(Wo[cs, :]).astype(ml_dtypes.bfloat16),
            cst=csT,
        )
        if use_mask:
            m["maskt"] = np.ascontiguousarray(mask[0, 0].T).astype(np.float32)
        in_maps.append(m)

    # transient device wedges (NRT_EXEC_UNIT_UNRECOVERABLE) clear on retry
    last_err = None
    for _attempt in range(3):
        try:
            res = bass_utils.run_bass_kernel_spmd(
                nc, in_maps, core_ids=list(range(NC)), trace=_trace)
            break
        except Exception as e:  # noqa: BLE001
            last_err = e
            import time as _time
            _time.sleep(2.0)
    else:
        raise last_err
    # row-parallel unshard: sum the per-core bf16 partials in fp32, add the
    # output bias and the folded V bias (bv commutes through attention).
    ypT = res.results[0]["ypT"].astype(np.float32)
    for c in range(1, NC):
        ypT = ypT + res.results[c]["ypT"].astype(np.float32)
    bo_eff = np.asarray(bo, dtype=np.float32) + \
        np.asarray(bv, dtype=np.float32) @ Wo
    out = (ypT.T + bo_eff).reshape(B, S, D).astype(np.float32)
    out = np.ascontiguousarray(out)
    if _trace:
        return out, res
    return out
